# Initial kernel scaffold
#
"""Contrastive-loss kernel for Trainium2, 8 NeuronCores (SPMD).

Math (reference): f = l2norm(features); sim = f@f.T / T; mask = labels equal;
loss = mean(-log(pos/(pos+neg+eps))) with pos = sum(sim*mask, 1),
neg = sum(sim*(1-mask), 1).

Factorization used here (avoids the N x N similarity matrix entirely):
  pos_i = (1/T) * f_i . C[label_i]      where C[c] = sum_{j: label_j=c} f_j
  pos_i + neg_i = (1/T) * f_i . g       where g = sum_j f_j = sum_c C[c]
  loss_i = -log(pos_i / (pos_i + neg_i + EPS_LOG))
         = log(f_i.g + T*EPS_LOG) - log(f_i.C[label_i])      (T*EPS_LOG = 1e-9)

Per-core work (rows sharded 8 ways, 2048 rows each):
  1. load row shard, row sumsq -> inv-norm (sqrt + reciprocal + Newton polish)
  2. one-hot(labels) via iota+is_equal; PE matmul accumulates local C^T [d, cls]
  3. transpose C^T -> C [cls, d], append g row; AllReduce over 8 cores
  4. indirect-DMA gather Q = C[labels]; fused dots P_i = (f_i*invn).Q_i,
     S_i = (f_i*invn).g ; loss partial = sum(ln(S+1e-9) - ln(P))  [NaN-exact]
  5. host: sum 8 partials / N
"""

import numpy as np

import concourse.bass as bass
import concourse.bacc as bacc
import concourse.mybir as mybir
import concourse.tile as tile
from concourse import bass_utils
from concourse.masks import make_identity

F32 = mybir.dt.float32
F16 = mybir.dt.float16
BF16 = mybir.dt.bfloat16
I32 = mybir.dt.int32
AF = mybir.ActivationFunctionType
OP = mybir.AluOpType

N, D, NC = 16384, 128, 8
NS = N // NC          # rows per core = 2048
P = 128               # partitions
T = NS // P           # row tiles per core = 16
NCLS = 1024           # classes padded to 1024 (real labels < 1000)


def build_program():
    nc = bacc.Bacc(
        "TRN2", target_bir_lowering=False, debug=False,
        enable_asserts=False, num_devices=NC,
    )

    feat = nc.dram_tensor("feat", [NS, D], F32, kind="ExternalInput")
    labf = nc.dram_tensor("labf", [P, T], F32, kind="ExternalInput")
    labi = nc.dram_tensor("labi", [P, T], I32, kind="ExternalInput")
    ploss = nc.dram_tensor("ploss", [1, 1], F32, kind="ExternalOutput")
    pdbg = nc.dram_tensor("pdbg", [P, T], F32, kind="ExternalOutput")
    sdbg = nc.dram_tensor("sdbg", [P, T], F32, kind="ExternalOutput")

    with tile.TileContext(nc) as tc:
        with (
            tc.tile_pool(name="sb", bufs=1) as sb,
            tc.tile_pool(name="oh_pool", bufs=3) as ohp,
            tc.tile_pool(name="q_pool", bufs=4) as qp,
            tc.tile_pool(name="scr_pool", bufs=2) as scrp,
            tc.tile_pool(name="ps", bufs=1, space="PSUM") as ps,
            tc.tile_pool(name="ps_tp", bufs=2, space="PSUM") as pstp,
            tc.tile_pool(name="dram", bufs=1, space="DRAM") as dram,
        ):
            # ---------- constants / setup ----------
            ident = sb.tile([P, P], F32)
            make_identity(nc, ident[:])
            eps_ln = sb.tile([P, 1], F32)
            nc.vector.memset(eps_ln[:], 1e-9)
            eps_n2 = sb.tile([P, 1], F32)
            nc.vector.memset(eps_n2[:], 1e-24)
            ones_row = sb.tile([1, P], F32)
            nc.vector.memset(ones_row[:], 1.0)
            ones_col = sb.tile([P, 1], F32)
            nc.vector.memset(ones_col[:], 1.0)

            # warm the sqrt ACT table set early (overlaps input DMA)
            dummy_a = sb.tile([1, 1], F32)
            nc.scalar.activation(dummy_a[:], eps_n2[0:1, 0:1], AF.Sqrt)

            iot32 = sb.tile([P, NCLS], I32)
            nc.gpsimd.iota(iot32[:], pattern=[[1, NCLS]], base=0,
                           channel_multiplier=0)
            iot16 = sb.tile([P, NCLS], F16)
            nc.vector.tensor_copy(iot16[:], iot32[:])

            # ---------- input loads ----------
            fsb = sb.tile([P, NS], F32)           # [p, t*128+d] = feat[t*128+p, d]
            nc.sync.dma_start(
                fsb[:].rearrange("p (t d) -> p t d", d=D),
                feat[:, :].rearrange("(t p) d -> p t d", p=P),
            )
            labft = sb.tile([P, T], F32)
            nc.sync.dma_start(labft[:], labf[:, :])
            labit = sb.tile([P, T], I32)
            nc.sync.dma_start(labit[:], labi[:, :])

            def ftile(t):
                return fsb[:, t * D:(t + 1) * D]

            # ---------- row norms ----------
            ssq = sb.tile([P, T], F32)
            for t in range(T):
                sq_scr = scrp.tile([P, D], BF16, tag="sq_scr")
                nc.gpsimd.scalar_tensor_tensor(
                    out=sq_scr[:], in0=ftile(t), scalar=1.0, in1=ftile(t),
                    op0=OP.mult, op1=OP.mult, accum_out=ssq[:, t:t + 1])

            nrm = sb.tile([P, T], F32)
            nc.scalar.activation(nrm[:], ssq[:], AF.Sqrt, bias=eps_n2[:, 0:1])
            inv0 = sb.tile([P, T], F32)
            nc.vector.reciprocal(inv0[:], nrm[:])
            # one Newton step for rsqrt(ssq): y1 = y0*(1.5 - 0.5*ssq*y0^2)
            nr_a = sb.tile([P, T], F32)
            nc.vector.tensor_tensor(nr_a[:], inv0[:], inv0[:], op=OP.mult)
            nc.vector.tensor_tensor(nr_a[:], nr_a[:], ssq[:], op=OP.mult)
            nc.vector.tensor_scalar(nr_a[:], nr_a[:], -0.5, 1.5,
                                    op0=OP.mult, op1=OP.add)
            invn = sb.tile([P, T], F32)
            nc.vector.tensor_tensor(invn[:], inv0[:], nr_a[:], op=OP.mult)

            # warm the ln ACT table set (dep on nrm keeps it after sqrt)
            dummy_b = sb.tile([1, 1], F32)
            nc.scalar.activation(dummy_b[:], nrm[0:1, 0:1], AF.Ln)

            # ---------- normalized bf16 features ----------
            fbf = sb.tile([P, NS], BF16)
            for t in range(T):
                nc.vector.tensor_scalar(
                    fbf[:, t * D:(t + 1) * D], ftile(t), invn[:, t:t + 1],
                    None, op0=OP.mult)

            # ---------- one-hot + local C^T accumulation on PE ----------
            psum_ct = ps.tile([P, NCLS], F32, space="PSUM")
            for t in range(T):
                oh = ohp.tile([P, NCLS], BF16, tag="oh")
                nc.vector.tensor_scalar(oh[:], iot16[:], labft[:, t:t + 1],
                                        None, op0=OP.is_equal)
                for h in range(2):
                    nc.tensor.matmul(
                        psum_ct[:, h * 512:(h + 1) * 512],
                        lhsT=fbf[:, t * D:(t + 1) * D],
                        rhs=oh[:, h * 512:(h + 1) * 512],
                        start=(t == 0), stop=(t == T - 1))

            # local g = rowwise sum over classes of C^T
            gl = sb.tile([P, 1], F32)
            nc.vector.tensor_reduce(gl[:], psum_ct[:], axis=mybir.AxisListType.X,
                                    op=OP.add)

            # ---------- transpose C^T -> C and ship to DRAM ----------
            ct_sb = sb.tile([P, NCLS], F32)
            nc.scalar.copy(ct_sb[:], psum_ct[:])

            bl_local = dram.tile([NCLS + 1, P], F32)
            bl_global = dram.tile([NCLS + 1, P], F32)
            for cch in range(NCLS // P):
                tp = pstp.tile([P, P], F32, space="PSUM", tag="tp")
                nc.tensor.transpose(tp[:], ct_sb[:, cch * P:(cch + 1) * P],
                                    ident[:])
                nc.sync.dma_start(bl_local[cch * P:(cch + 1) * P, :], tp[:])
            nc.sync.dma_start(
                bl_local[NCLS:NCLS + 1, :].rearrange("one d -> d one"), gl[:])

            # ---------- AllReduce ----------
            nc.gpsimd.collective_compute(
                "AllReduce", OP.add,
                replica_groups=[list(range(NC))],
                ins=[bl_local.opt()],
                outs=[bl_global.opt()])

            # ---------- g broadcast [P, D] via K=1 ones-matmul ----------
            grow = sb.tile([1, D], F32)
            nc.sync.dma_start(grow[:], bl_global[NCLS:NCLS + 1, :])
            psum_gb = ps.tile([P, D], F32, space="PSUM")
            nc.tensor.matmul(psum_gb[:], lhsT=ones_row[:], rhs=grow[:],
                             start=True, stop=True)
            gb = sb.tile([P, D], F32)
            nc.scalar.copy(gb[:], psum_gb[:])

            # ---------- gather Q = C[labels] and fused dot products ----------
            pcol = sb.tile([P, T], F32)
            scol = sb.tile([P, T], F32)
            for t in range(T):
                q = qp.tile([P, D], F32, tag="q")
                nc.gpsimd.indirect_dma_start(
                    out=q[:], out_offset=None, in_=bl_global[:, :],
                    in_offset=bass.IndirectOffsetOnAxis(
                        ap=labit[:, t:t + 1], axis=0))
                p_scr = scrp.tile([P, D], BF16, tag="p_scr")
                nc.vector.scalar_tensor_tensor(
                    out=p_scr[:], in0=ftile(t), scalar=invn[:, t:t + 1],
                    in1=q[:], op0=OP.mult, op1=OP.mult,
                    accum_out=pcol[:, t:t + 1])
                s_scr = scrp.tile([P, D], BF16, tag="s_scr")
                nc.gpsimd.scalar_tensor_tensor(
                    out=s_scr[:], in0=ftile(t), scalar=invn[:, t:t + 1],
                    in1=gb[:], op0=OP.mult, op1=OP.mult,
                    accum_out=scol[:, t:t + 1])

            nc.sync.dma_start(pdbg[:, :], pcol[:])
            nc.sync.dma_start(sdbg[:, :], scol[:])

            # ---------- loss = sum(ln(S + 1e-9) - ln(P)) ----------
            ln_s = sb.tile([P, T], F32)
            nc.scalar.activation(ln_s[:], scol[:], AF.Ln, bias=eps_ln[:, 0:1])
            ln_p = sb.tile([P, T], F32)
            nc.scalar.activation(ln_p[:], pcol[:], AF.Ln)
            diff = sb.tile([P, T], F32)
            nc.vector.tensor_tensor(diff[:], ln_s[:], ln_p[:], op=OP.subtract)
            rowp = sb.tile([P, 1], F32)
            nc.vector.tensor_reduce(rowp[:], diff[:], axis=mybir.AxisListType.X,
                                    op=OP.add)
            psum_s = ps.tile([1, 1], F32, space="PSUM")
            nc.tensor.matmul(psum_s[:], lhsT=rowp[:], rhs=ones_col[:],
                             start=True, stop=True)
            sout = sb.tile([1, 1], F32)
            nc.scalar.copy(sout[:], psum_s[:])
            nc.sync.dma_start(ploss[:, :], sout[:])

    nc.compile()
    return nc


_PROG = None


def _get_prog():
    global _PROG
    if _PROG is None:
        _PROG = build_program()
    return _PROG


def make_in_maps(features, labels):
    features = np.ascontiguousarray(np.asarray(features, dtype=np.float32))
    labels = np.asarray(labels)
    assert features.shape == (N, D), features.shape
    assert labels.shape == (N,), labels.shape
    in_maps = []
    for c in range(NC):
        fs = features[c * NS:(c + 1) * NS]
        ls = labels[c * NS:(c + 1) * NS].reshape(T, P).T   # [P, T]
        in_maps.append(dict(
            feat=np.ascontiguousarray(fs),
            labf=np.ascontiguousarray(ls.astype(np.float32)),
            labi=np.ascontiguousarray(ls.astype(np.int32)),
        ))
    return in_maps


def run(features, labels, trace=False):
    nc = _get_prog()
    res = bass_utils.run_bass_kernel_spmd(
        nc, make_in_maps(features, labels),
        core_ids=list(range(NC)), trace=trace)
    total = np.float64(0.0)
    for c in range(NC):
        total += np.float64(res.results[c]["ploss"][0, 0])
    out = np.asarray(np.float32(total / N))
    return out, res


def kernel(features, labels):
    out, _ = run(features, labels)
    return out


# revision 4
# speedup vs baseline: 2.0209x; 2.0209x over previous
"""Contrastive-loss kernel for Trainium2, 8 NeuronCores (SPMD).

Math (reference): f = l2norm(features); sim = f@f.T / T; mask = labels equal;
loss = mean(-log(pos/(pos+neg+eps))) with pos = sum(sim*mask, 1),
neg = sum(sim*(1-mask), 1).

Factorization used here (avoids the N x N similarity matrix entirely):
  pos_i = (1/T) * f_i . C[label_i]      where C[c] = sum_{j: label_j=c} f_j
  pos_i + neg_i = (1/T) * f_i . g       where g = sum_j f_j = sum_c C[c]
  loss_i = -log(pos_i / (pos_i + neg_i + EPS_LOG))
         = log(f_i.g + T*EPS_LOG) - log(f_i.C[label_i])      (T*EPS_LOG = 1e-9)

Per-core work (rows sharded 8 ways, 2048 rows each):
  1. load row shard, row sumsq -> inv-norm (sqrt + reciprocal + Newton polish)
  2. one-hot(labels) via iota+is_equal; PE matmul accumulates local C^T [d, cls]
  3. transpose C^T -> C [cls, d], append g row; AllReduce over 8 cores
  4. indirect-DMA gather Q = C[labels]; fused dots P_i = (f_i*invn).Q_i,
     S_i = (f_i*invn).g ; loss partial = sum(ln(S+1e-9) - ln(P))  [NaN-exact]
  5. host: sum 8 partials / N
"""

import numpy as np

import concourse.bass as bass
import concourse.bacc as bacc
import concourse.mybir as mybir
import concourse.tile as tile
from concourse import bass_utils
from concourse.masks import make_identity

F32 = mybir.dt.float32
F16 = mybir.dt.float16
BF16 = mybir.dt.bfloat16
I32 = mybir.dt.int32
AF = mybir.ActivationFunctionType
OP = mybir.AluOpType

N, D, NC = 16384, 128, 8
NS = N // NC          # rows per core = 2048
P = 128               # partitions
T = NS // P           # row tiles per core = 16
NCLS = 1024           # classes padded to 1024 (real labels < 1000)


def build_program(repeats=1, debug_outs=True):
    nc = bacc.Bacc(
        "TRN2", target_bir_lowering=False, debug=False,
        enable_asserts=False, num_devices=NC,
    )

    feat = nc.dram_tensor("feat", [NS, D], F32, kind="ExternalInput")
    labf = nc.dram_tensor("labf", [P, T], F32, kind="ExternalInput")
    labi = nc.dram_tensor("labi", [P, T], I32, kind="ExternalInput")
    plosses = [nc.dram_tensor(f"ploss{r}", [1, 1], F32, kind="ExternalOutput")
               for r in range(repeats)]
    if debug_outs:
        pdbg = nc.dram_tensor("pdbg", [P, T], F32, kind="ExternalOutput")
        sdbg = nc.dram_tensor("sdbg", [P, T], F32, kind="ExternalOutput")

    with tile.TileContext(nc) as tc:
      for rep in range(repeats):
        ploss = plosses[rep]
        emit_debug = debug_outs and rep == 0
        with (
            tc.tile_pool(name="sb", bufs=1) as sb,
            tc.tile_pool(name="oh_pool", bufs=3) as ohp,
            tc.tile_pool(name="q_pool", bufs=4) as qp,
            tc.tile_pool(name="scr_pool", bufs=2) as scrp,
            tc.tile_pool(name="ps", bufs=1, space="PSUM") as ps,
            tc.tile_pool(name="ps_tp", bufs=2, space="PSUM") as pstp,
            tc.tile_pool(name="dram", bufs=1, space="DRAM") as dram,
        ):
            # ---------- constants / setup ----------
            ident = sb.tile([P, P], F32)
            make_identity(nc, ident[:])
            eps_ln = sb.tile([P, 1], F32)
            nc.vector.memset(eps_ln[:], 1e-9)
            eps_n2 = sb.tile([P, 1], F32)
            nc.vector.memset(eps_n2[:], 1e-24)
            ones_row = sb.tile([1, P], F32)
            nc.vector.memset(ones_row[:], 1.0)
            ones_col = sb.tile([P, 1], F32)
            nc.vector.memset(ones_col[:], 1.0)

            # warm the sqrt ACT table set early (overlaps input DMA)
            dummy_a = sb.tile([1, 1], F32)
            nc.scalar.activation(dummy_a[:], eps_n2[0:1, 0:1], AF.Sqrt)

            iot32 = sb.tile([P, NCLS], I32)
            nc.gpsimd.iota(iot32[:], pattern=[[1, NCLS]], base=0,
                           channel_multiplier=0)
            iot16 = sb.tile([P, NCLS], F16)
            nc.vector.tensor_copy(iot16[:], iot32[:])

            # ---------- input loads ----------
            fsb = sb.tile([P, NS], F32)           # [p, t*128+d] = feat[t*128+p, d]
            nc.sync.dma_start(
                fsb[:].rearrange("p (t d) -> p t d", d=D),
                feat[:, :].rearrange("(t p) d -> p t d", p=P),
            )
            labft = sb.tile([P, T], F32)
            nc.sync.dma_start(labft[:], labf[:, :])
            labit = sb.tile([P, T], I32)
            nc.sync.dma_start(labit[:], labi[:, :])

            def ftile(t):
                return fsb[:, t * D:(t + 1) * D]

            # ---------- row norms ----------
            ssq = sb.tile([P, T], F32)
            for t in range(T):
                sq_scr = scrp.tile([P, D], BF16, tag="sq_scr")
                nc.vector.scalar_tensor_tensor(
                    out=sq_scr[:], in0=ftile(t), scalar=1.0, in1=ftile(t),
                    op0=OP.mult, op1=OP.mult, accum_out=ssq[:, t:t + 1])

            nrm = sb.tile([P, T], F32)
            nc.scalar.activation(nrm[:], ssq[:], AF.Sqrt, bias=eps_n2[:, 0:1])
            inv0 = sb.tile([P, T], F32)
            nc.vector.reciprocal(inv0[:], nrm[:])
            # one Newton step for rsqrt(ssq): y1 = y0*(1.5 - 0.5*ssq*y0^2)
            nr_a = sb.tile([P, T], F32)
            nc.vector.tensor_tensor(nr_a[:], inv0[:], inv0[:], op=OP.mult)
            nc.vector.tensor_tensor(nr_a[:], nr_a[:], ssq[:], op=OP.mult)
            nc.vector.tensor_scalar(nr_a[:], nr_a[:], -0.5, 1.5,
                                    op0=OP.mult, op1=OP.add)
            invn = sb.tile([P, T], F32)
            nc.vector.tensor_tensor(invn[:], inv0[:], nr_a[:], op=OP.mult)

            # warm the ln ACT table set (dep on nrm keeps it after sqrt)
            dummy_b = sb.tile([1, 1], F32)
            nc.scalar.activation(dummy_b[:], nrm[0:1, 0:1], AF.Ln)

            # ---------- normalized bf16 features ----------
            fbf = sb.tile([P, NS], BF16)
            for t in range(T):
                nc.vector.tensor_scalar(
                    fbf[:, t * D:(t + 1) * D], ftile(t), invn[:, t:t + 1],
                    None, op0=OP.mult)

            # ---------- one-hot + local C^T accumulation on PE ----------
            psum_ct = ps.tile([P, NCLS], F32, space="PSUM")
            for t in range(T):
                oh = ohp.tile([P, NCLS], BF16, tag="oh")
                nc.vector.tensor_scalar(oh[:], iot16[:], labft[:, t:t + 1],
                                        None, op0=OP.is_equal)
                for h in range(2):
                    nc.tensor.matmul(
                        psum_ct[:, h * 512:(h + 1) * 512],
                        lhsT=fbf[:, t * D:(t + 1) * D],
                        rhs=oh[:, h * 512:(h + 1) * 512],
                        start=(t == 0), stop=(t == T - 1))

            # local g = rowwise sum over classes of C^T
            gl = sb.tile([P, 1], F32)
            nc.vector.tensor_reduce(gl[:], psum_ct[:], axis=mybir.AxisListType.X,
                                    op=OP.add)

            # ---------- transpose C^T -> C and ship to DRAM ----------
            ct_sb = sb.tile([P, NCLS], F32)
            nc.scalar.copy(ct_sb[:], psum_ct[:])

            bl_local = dram.tile([NCLS + 1, P], F32)
            bl_global = dram.tile([NCLS + 1, P], F32)
            for cch in range(NCLS // P):
                tp = pstp.tile([P, P], F32, space="PSUM", tag="tp")
                nc.tensor.transpose(tp[:], ct_sb[:, cch * P:(cch + 1) * P],
                                    ident[:])
                c_chunk = scrp.tile([P, P], F32, tag="c_chunk")
                nc.vector.tensor_copy(c_chunk[:], tp[:])
                nc.sync.dma_start(bl_local[cch * P:(cch + 1) * P, :],
                                  c_chunk[:])
            nc.sync.dma_start(
                bl_local[NCLS:NCLS + 1, :].rearrange("one d -> d one"), gl[:])

            # ---------- AllReduce ----------
            nc.gpsimd.collective_compute(
                "AllReduce", OP.add,
                replica_groups=[list(range(NC))],
                ins=[bl_local.opt()],
                outs=[bl_global.opt()])

            # ---------- g broadcast [P, D] via K=1 ones-matmul ----------
            grow = sb.tile([1, D], F32)
            nc.sync.dma_start(grow[:], bl_global[NCLS:NCLS + 1, :])
            psum_gb = ps.tile([P, D], F32, space="PSUM")
            nc.tensor.matmul(psum_gb[:], lhsT=ones_row[:], rhs=grow[:],
                             start=True, stop=True)
            gb = sb.tile([P, D], F32)
            nc.scalar.copy(gb[:], psum_gb[:])

            # ---------- gather Q = C[labels] and fused dot products ----------
            pcol = sb.tile([P, T], F32)
            scol = sb.tile([P, T], F32)
            for t in range(T):
                q = qp.tile([P, D], F32, tag="q")
                nc.gpsimd.indirect_dma_start(
                    out=q[:], out_offset=None, in_=bl_global[:, :],
                    in_offset=bass.IndirectOffsetOnAxis(
                        ap=labit[:, t:t + 1], axis=0))
                p_scr = scrp.tile([P, D], BF16, tag="p_scr")
                nc.vector.scalar_tensor_tensor(
                    out=p_scr[:], in0=ftile(t), scalar=invn[:, t:t + 1],
                    in1=q[:], op0=OP.mult, op1=OP.mult,
                    accum_out=pcol[:, t:t + 1])
                s_scr = scrp.tile([P, D], BF16, tag="s_scr")
                nc.vector.scalar_tensor_tensor(
                    out=s_scr[:], in0=ftile(t), scalar=invn[:, t:t + 1],
                    in1=gb[:], op0=OP.mult, op1=OP.mult,
                    accum_out=scol[:, t:t + 1])

            if emit_debug:
                nc.sync.dma_start(pdbg[:, :], pcol[:])
                nc.sync.dma_start(sdbg[:, :], scol[:])

            # ---------- loss = sum(ln(S + 1e-9) - ln(P)) ----------
            ln_s = sb.tile([P, T], F32)
            nc.scalar.activation(ln_s[:], scol[:], AF.Ln, bias=eps_ln[:, 0:1])
            ln_p = sb.tile([P, T], F32)
            nc.scalar.activation(ln_p[:], pcol[:], AF.Ln)
            diff = sb.tile([P, T], F32)
            nc.vector.tensor_tensor(diff[:], ln_s[:], ln_p[:], op=OP.subtract)
            rowp = sb.tile([P, 1], F32)
            nc.vector.tensor_reduce(rowp[:], diff[:], axis=mybir.AxisListType.X,
                                    op=OP.add)
            psum_s = ps.tile([1, 1], F32, space="PSUM")
            nc.tensor.matmul(psum_s[:], lhsT=rowp[:], rhs=ones_col[:],
                             start=True, stop=True)
            sout = sb.tile([1, 1], F32)
            nc.scalar.copy(sout[:], psum_s[:])
            nc.sync.dma_start(ploss[:, :], sout[:])

    nc.compile()
    return nc


_PROG = None


def _get_prog():
    global _PROG
    if _PROG is None:
        _PROG = build_program()
    return _PROG


def make_in_maps(features, labels):
    features = np.ascontiguousarray(np.asarray(features, dtype=np.float32))
    labels = np.asarray(labels)
    assert features.shape == (N, D), features.shape
    assert labels.shape == (N,), labels.shape
    in_maps = []
    for c in range(NC):
        fs = features[c * NS:(c + 1) * NS]
        ls = labels[c * NS:(c + 1) * NS].reshape(T, P).T   # [P, T]
        in_maps.append(dict(
            feat=np.ascontiguousarray(fs),
            labf=np.ascontiguousarray(ls.astype(np.float32)),
            labi=np.ascontiguousarray(ls.astype(np.int32)),
        ))
    return in_maps


def run(features, labels, trace=False):
    nc = _get_prog()
    res = bass_utils.run_bass_kernel_spmd(
        nc, make_in_maps(features, labels),
        core_ids=list(range(NC)), trace=trace)
    total = np.float64(0.0)
    for c in range(NC):
        total += np.float64(res.results[c]["ploss0"][0, 0])
    out = np.asarray(np.float32(total / N))
    return out, res


def kernel(features, labels):
    out, _ = run(features, labels)
    return out


# revision 5
# speedup vs baseline: 7.0976x; 3.5121x over previous
"""Contrastive-loss kernel for Trainium2, 8 NeuronCores (SPMD).

Math (reference): f = l2norm(features); sim = f@f.T / T; mask = labels equal;
loss = mean(-log(pos/(pos+neg+eps))) with pos = sum(sim*mask, 1),
neg = sum(sim*(1-mask), 1).

Factorization used here (avoids the N x N similarity matrix entirely):
  pos_i = (1/T) * f_i . C[label_i]      where C[c] = sum_{j: label_j=c} f_j
  pos_i + neg_i = (1/T) * f_i . g       where g = sum_j f_j = sum_c C[c]
  loss_i = -log(pos_i / (pos_i + neg_i + EPS_LOG))
         = log(f_i.g + T*EPS_LOG) - log(f_i.C[label_i])      (T*EPS_LOG = 1e-9)

Per-core work (rows sharded 8 ways, 2048 rows each):
  1. load row shard, row sumsq -> inv-norm (sqrt + reciprocal + Newton polish)
  2. one-hot(labels) via iota+is_equal; PE matmul accumulates local C^T [d, cls]
  3. transpose C^T -> C [cls, d], append g row; AllReduce over 8 cores
  4. indirect-DMA gather Q = C[labels]; fused dots P_i = (f_i*invn).Q_i,
     S_i = (f_i*invn).g ; loss partial = sum(ln(S+1e-9) - ln(P))  [NaN-exact]
  5. host: sum 8 partials / N
"""

import numpy as np

import concourse.bass as bass
import concourse.bacc as bacc
import concourse.mybir as mybir
import concourse.tile as tile
from concourse import bass_utils
from concourse.masks import make_identity

F32 = mybir.dt.float32
F16 = mybir.dt.float16
BF16 = mybir.dt.bfloat16
I32 = mybir.dt.int32
AF = mybir.ActivationFunctionType
OP = mybir.AluOpType

N, D, NC = 16384, 128, 8
NS = N // NC          # rows per core = 2048
P = 128               # partitions
T = NS // P           # row tiles per core = 16
NCLS = 1024           # classes padded to 1024 (real labels < 1000)


def build_program(repeats=1, debug_outs=True, variant="full"):
    nc = bacc.Bacc(
        "TRN2", target_bir_lowering=False, debug=False,
        enable_asserts=False, num_devices=NC,
    )

    feat = nc.dram_tensor("feat", [NS, D], F32, kind="ExternalInput")
    labf = nc.dram_tensor("labf", [P, T], F32, kind="ExternalInput")
    labi = nc.dram_tensor("labi", [P, T], I32, kind="ExternalInput")
    plosses = [nc.dram_tensor(f"ploss{r}", [1, 1], F32, kind="ExternalOutput")
               for r in range(repeats)]
    if debug_outs:
        pdbg = nc.dram_tensor("pdbg", [P, T], F32, kind="ExternalOutput")
        sdbg = nc.dram_tensor("sdbg", [P, T], F32, kind="ExternalOutput")

    with tile.TileContext(nc) as tc:
      for rep in range(repeats):
        ploss = plosses[rep]
        emit_debug = debug_outs and rep == 0
        with (
            tc.tile_pool(name="sb", bufs=1) as sb,
            tc.tile_pool(name="oh_pool", bufs=3) as ohp,
            tc.tile_pool(name="q_pool", bufs=4) as qp,
            tc.tile_pool(name="scr_pool", bufs=2) as scrp,
            tc.tile_pool(name="ps", bufs=1, space="PSUM") as ps,
            tc.tile_pool(name="ps_tp", bufs=2, space="PSUM") as pstp,
            tc.tile_pool(name="dram", bufs=1, space="DRAM") as dram,
        ):
            # ---------- constants / setup ----------
            ident = sb.tile([P, P], F32)
            make_identity(nc, ident[:])
            eps_ln = sb.tile([P, 1], F32)
            nc.vector.memset(eps_ln[:], 1e-9)
            eps_n2 = sb.tile([P, 1], F32)
            nc.vector.memset(eps_n2[:], 1e-24)
            ones_row = sb.tile([1, P], F32)
            nc.vector.memset(ones_row[:], 1.0)
            ones_col = sb.tile([P, 1], F32)
            nc.vector.memset(ones_col[:], 1.0)

            # warm the sqrt ACT table set early (overlaps input DMA)
            dummy_a = sb.tile([1, 1], F32)
            nc.scalar.activation(dummy_a[:], eps_n2[0:1, 0:1], AF.Sqrt)

            iot32 = sb.tile([P, NCLS], I32)
            nc.gpsimd.iota(iot32[:], pattern=[[1, NCLS]], base=0,
                           channel_multiplier=0)
            iot16 = sb.tile([P, NCLS], F16)
            nc.vector.tensor_copy(iot16[:], iot32[:])

            # ---------- input loads ----------
            fsb = sb.tile([P, NS], F32)           # [p, t*128+d] = feat[t*128+p, d]
            nc.sync.dma_start(
                fsb[:].rearrange("p (t d) -> p t d", d=D),
                feat[:, :].rearrange("(t p) d -> p t d", p=P),
            )
            labft = sb.tile([P, T], F32)
            nc.sync.dma_start(labft[:], labf[:, :])
            labit = sb.tile([P, T], I32)
            nc.sync.dma_start(labit[:], labi[:, :])

            def ftile(t):
                return fsb[:, t * D:(t + 1) * D]

            # ---------- row norms ----------
            ssq = sb.tile([P, T], F32)
            for t in range(T):
                sq_scr = scrp.tile([P, D], BF16, tag="sq_scr")
                nc.vector.scalar_tensor_tensor(
                    out=sq_scr[:], in0=ftile(t), scalar=1.0, in1=ftile(t),
                    op0=OP.mult, op1=OP.mult, accum_out=ssq[:, t:t + 1])

            nrm = sb.tile([P, T], F32)
            nc.scalar.activation(nrm[:], ssq[:], AF.Sqrt, bias=eps_n2[:, 0:1])
            inv0 = sb.tile([P, T], F32)
            nc.vector.reciprocal(inv0[:], nrm[:])
            # one Newton step for rsqrt(ssq): y1 = y0*(1.5 - 0.5*ssq*y0^2)
            nr_a = sb.tile([P, T], F32)
            nc.vector.tensor_tensor(nr_a[:], inv0[:], inv0[:], op=OP.mult)
            nc.vector.tensor_tensor(nr_a[:], nr_a[:], ssq[:], op=OP.mult)
            nc.vector.tensor_scalar(nr_a[:], nr_a[:], -0.5, 1.5,
                                    op0=OP.mult, op1=OP.add)
            invn = sb.tile([P, T], F32)
            nc.vector.tensor_tensor(invn[:], inv0[:], nr_a[:], op=OP.mult)

            # warm the ln ACT table set (dep on nrm keeps it after sqrt)
            dummy_b = sb.tile([1, 1], F32)
            nc.scalar.activation(dummy_b[:], nrm[0:1, 0:1], AF.Ln)

            # ---------- normalized bf16 features ----------
            fbf = sb.tile([P, NS], BF16)
            for t in range(T):
                nc.vector.tensor_scalar(
                    fbf[:, t * D:(t + 1) * D], ftile(t), invn[:, t:t + 1],
                    None, op0=OP.mult)

            # ---------- one-hot + local C^T accumulation on PE ----------
            psum_ct = ps.tile([P, NCLS], F32, space="PSUM")
            for t in range(T):
                oh = ohp.tile([P, NCLS], BF16, tag="oh")
                nc.vector.tensor_scalar(oh[:], iot16[:], labft[:, t:t + 1],
                                        None, op0=OP.is_equal)
                for h in range(2):
                    nc.tensor.matmul(
                        psum_ct[:, h * 512:(h + 1) * 512],
                        lhsT=fbf[:, t * D:(t + 1) * D],
                        rhs=oh[:, h * 512:(h + 1) * 512],
                        start=(t == 0), stop=(t == T - 1))

            # local g = rowwise sum over classes of C^T
            gl = sb.tile([P, 1], F32)
            nc.vector.tensor_reduce(gl[:], psum_ct[:], axis=mybir.AxisListType.X,
                                    op=OP.add)

            # ---------- transpose C^T -> C and ship to DRAM ----------
            ct_sb = sb.tile([P, NCLS], F32)
            nc.scalar.copy(ct_sb[:], psum_ct[:])

            bl_local = dram.tile([NCLS + 1, P], F32)
            bl_global = dram.tile([NCLS + 1, P], F32)
            for cch in range(NCLS // P):
                tp = pstp.tile([P, P], F32, space="PSUM", tag="tp")
                nc.tensor.transpose(tp[:], ct_sb[:, cch * P:(cch + 1) * P],
                                    ident[:])
                c_chunk = scrp.tile([P, P], F32, tag="c_chunk")
                nc.vector.tensor_copy(c_chunk[:], tp[:])
                nc.sync.dma_start(bl_local[cch * P:(cch + 1) * P, :],
                                  c_chunk[:])
            nc.sync.dma_start(
                bl_local[NCLS:NCLS + 1, :].rearrange("one d -> d one"), gl[:])

            # ---------- AllReduce ----------
            if variant in ("full", "nogather"):
                nc.gpsimd.collective_compute(
                    "AllReduce", OP.add,
                    replica_groups=[list(range(NC))],
                    ins=[bl_local.opt()],
                    outs=[bl_global.opt()])
            else:
                bl_global = bl_local

            # ---------- g broadcast [P, D] via K=1 ones-matmul ----------
            grow = sb.tile([1, D], F32)
            nc.sync.dma_start(grow[:], bl_global[NCLS:NCLS + 1, :])
            psum_gb = ps.tile([P, D], F32, space="PSUM")
            nc.tensor.matmul(psum_gb[:], lhsT=ones_row[:], rhs=grow[:],
                             start=True, stop=True)
            gb = sb.tile([P, D], F32)
            nc.scalar.copy(gb[:], psum_gb[:])

            # ---------- gather Q = C[labels] and fused dot products ----------
            pcol = sb.tile([P, T], F32)
            scol = sb.tile([P, T], F32)
            for t in range(T):
                q = qp.tile([P, D], F32, tag="q")
                if variant in ("full", "nocoll"):
                    nc.gpsimd.indirect_dma_start(
                        out=q[:], out_offset=None, in_=bl_global[:, :],
                        in_offset=bass.IndirectOffsetOnAxis(
                            ap=labit[:, t:t + 1], axis=0))
                else:
                    nc.sync.dma_start(q[:], bl_global[0:P, :])
                p_scr = scrp.tile([P, D], BF16, tag="p_scr")
                nc.vector.scalar_tensor_tensor(
                    out=p_scr[:], in0=ftile(t), scalar=invn[:, t:t + 1],
                    in1=q[:], op0=OP.mult, op1=OP.mult,
                    accum_out=pcol[:, t:t + 1])
                s_scr = scrp.tile([P, D], BF16, tag="s_scr")
                nc.vector.scalar_tensor_tensor(
                    out=s_scr[:], in0=ftile(t), scalar=invn[:, t:t + 1],
                    in1=gb[:], op0=OP.mult, op1=OP.mult,
                    accum_out=scol[:, t:t + 1])

            if emit_debug:
                nc.sync.dma_start(pdbg[:, :], pcol[:])
                nc.sync.dma_start(sdbg[:, :], scol[:])

            # ---------- loss = sum(ln(S + 1e-9) - ln(P)) ----------
            ln_s = sb.tile([P, T], F32)
            nc.scalar.activation(ln_s[:], scol[:], AF.Ln, bias=eps_ln[:, 0:1])
            ln_p = sb.tile([P, T], F32)
            nc.scalar.activation(ln_p[:], pcol[:], AF.Ln)
            diff = sb.tile([P, T], F32)
            nc.vector.tensor_tensor(diff[:], ln_s[:], ln_p[:], op=OP.subtract)
            rowp = sb.tile([P, 1], F32)
            nc.vector.tensor_reduce(rowp[:], diff[:], axis=mybir.AxisListType.X,
                                    op=OP.add)
            psum_s = ps.tile([1, 1], F32, space="PSUM")
            nc.tensor.matmul(psum_s[:], lhsT=rowp[:], rhs=ones_col[:],
                             start=True, stop=True)
            sout = sb.tile([1, 1], F32)
            nc.scalar.copy(sout[:], psum_s[:])
            nc.sync.dma_start(ploss[:, :], sout[:])

    nc.compile()
    return nc


_PROG = None


def _get_prog():
    global _PROG
    if _PROG is None:
        _PROG = build_program()
    return _PROG


def make_in_maps(features, labels):
    features = np.ascontiguousarray(np.asarray(features, dtype=np.float32))
    labels = np.asarray(labels)
    assert features.shape == (N, D), features.shape
    assert labels.shape == (N,), labels.shape
    in_maps = []
    for c in range(NC):
        fs = features[c * NS:(c + 1) * NS]
        ls = labels[c * NS:(c + 1) * NS].reshape(T, P).T   # [P, T]
        in_maps.append(dict(
            feat=np.ascontiguousarray(fs),
            labf=np.ascontiguousarray(ls.astype(np.float32)),
            labi=np.ascontiguousarray(ls.astype(np.int32)),
        ))
    return in_maps


def run(features, labels, trace=False):
    nc = _get_prog()
    res = bass_utils.run_bass_kernel_spmd(
        nc, make_in_maps(features, labels),
        core_ids=list(range(NC)), trace=trace)
    total = np.float64(0.0)
    for c in range(NC):
        total += np.float64(res.results[c]["ploss0"][0, 0])
    out = np.asarray(np.float32(total / N))
    return out, res


def kernel(features, labels):
    out, _ = run(features, labels)
    return out
